# revision 9
# baseline (speedup 1.0000x reference)
"""MoE SwiGLU (T=4096, D=I=1024, E=8, top-2) on 8 Trainium2 NeuronCores.

Expert-parallel with on-device routing: core e holds expert e's weights
in SBUF.  The gate (scores -> softmax -> top-2) is replicated on every
core in true fp32.  Each core then COMPACTS the token ids routed to its
expert (matmul prefix-sums + indirect scatter), gathers just those x
rows (indirect DMA), computes SwiGLU only for them (float32r matmuls at
full PE rate), scales by the routing weight, and scatters the rows into
a zeroed per-range contribution buffer.  Four token-range ReduceScatters
overlap compute; the host reassembles the 8 shards.

Work is organized in 4 token ranges of 1024; per (core, range) the
routed token count is ~256 +- 14 (capacity 384, overflow checked on the
host against the actual gate before launch).
"""
import os
import sys

import numpy as np

for _p in ("/opt/trn_rl_repo", "/root/.axon_site/_ro/trn_rl_repo"):
    if os.path.isdir(_p) and _p not in sys.path:
        sys.path.append(_p)

import concourse.bass as bass  # noqa: E402
import concourse.mybir as mybir  # noqa: E402
import concourse.tile as tile  # noqa: E402
from concourse import bacc  # noqa: E402
from concourse.bass_utils import run_bass_kernel_spmd  # noqa: E402

P = 128
T, D, I, E, TOPK = 4096, 1024, 1024, 8, 2
NCORES = 8
TCH = 512            # gate token chunk (matmul free dim)
NCH = T // TCH       # 8
DK = D // P          # 8
IK = I // P          # 8
NQ = 4               # ReduceScatter ranges
RT = T // NQ         # 1024 tokens per range
RSH = RT // NCORES   # 128-token shard per core per range
CAP = 384            # routed-token capacity per (core, range)
CT = CAP // P        # 3 c-tiles per range
YC_ROWS = RT + P     # contribution rows + trash row region
XPAD_ROWS = T + P    # x padded with zero rows (gather trash target)
f32 = mybir.dt.float32
f32r = mybir.dt.float32r
i32 = mybir.dt.int32

_CACHED_NC = None


def _build():
    nc = bacc.Bacc("TRN2", target_bir_lowering=False, debug=False,
                   num_devices=NCORES)
    xT_d = nc.dram_tensor("xT", [D, T], f32, kind="ExternalInput")
    x_d = nc.dram_tensor("x", [XPAD_ROWS, D], f32r, kind="ExternalInput")
    gwT_d = nc.dram_tensor("gwT", [D, E], f32, kind="ExternalInput")
    w1T_d = nc.dram_tensor("w1T", [D, I], f32r, kind="ExternalInput")
    w3T_d = nc.dram_tensor("w3T", [D, I], f32r, kind="ExternalInput")
    w2T_d = nc.dram_tensor("w2T", [I, D], f32r, kind="ExternalInput")
    utri_d = nc.dram_tensor("utri", [P, P], f32, kind="ExternalInput")
    ones_d = nc.dram_tensor("ones", [P, P], f32, kind="ExternalInput")
    ident_d = nc.dram_tensor("ident", [P, P], f32r, kind="ExternalInput")
    tidb_d = nc.dram_tensor("tidb", [P, E], f32, kind="ExternalInput")
    y_d = nc.dram_tensor("y", [NQ * RSH, D], f32, kind="ExternalOutput")

    with tile.TileContext(nc) as tc:
        with tc.tile_pool(name="wpool", bufs=1) as wpool, \
             tc.tile_pool(name="xgpool", bufs=1) as xgpool, \
             tc.tile_pool(name="gpool", bufs=2) as gpool, \
             tc.tile_pool(name="wapool", bufs=4) as wapool, \
             tc.tile_pool(name="cpool", bufs=2) as cpool, \
             tc.tile_pool(name="xepool", bufs=3) as xepool, \
             tc.tile_pool(name="xtpool", bufs=2) as xtpool, \
             tc.tile_pool(name="apool", bufs=2) as apool, \
             tc.tile_pool(name="spool", bufs=2) as spool, \
             tc.tile_pool(name="ypool", bufs=2) as ypool, \
             tc.tile_pool(name="psum", bufs=2, space="PSUM") as psum, \
             tc.tile_pool(name="pyps", bufs=2, space="PSUM") as pyps, \
             tc.tile_pool(name="psmall", bufs=2, space="PSUM") as psmall, \
             tc.tile_pool(name="dram", bufs=1, space="DRAM") as dram:

            # --- constants + resident weights ---
            gwT_s = wpool.tile([P, DK, E], f32, tag="gw")
            nc.sync.dma_start(gwT_s[:], gwT_d[:, :].rearrange("(o p) e -> p o e", p=P))
            utri_s = wpool.tile([P, P], f32, tag="utri")
            nc.sync.dma_start(utri_s[:], utri_d[:, :])
            ones_s = wpool.tile([P, P], f32, tag="ones")
            nc.sync.dma_start(ones_s[:], ones_d[:, :])
            ident_s = wpool.tile([P, P], f32r, tag="ident")
            nc.sync.dma_start(ident_s[:], ident_d[:, :])
            tidb_s = wpool.tile([P, E], f32, tag="tidb")
            nc.sync.dma_start(tidb_s[:], tidb_d[:, :])

            w1T_s = wpool.tile([P, DK, I], f32r, tag="w1")
            w3T_s = wpool.tile([P, DK, I], f32r, tag="w3")
            w2T_s = wpool.tile([P, IK, D], f32r, tag="w2")
            for h in range(4):
                hs = slice(h * (I // 4), (h + 1) * (I // 4))
                nc.sync.dma_start(
                    w1T_s[:, :, hs], w1T_d[:, hs].rearrange("(o p) i -> p o i", p=P))
                nc.sync.dma_start(
                    w3T_s[:, :, hs], w3T_d[:, hs].rearrange("(o p) i -> p o i", p=P))
                nc.sync.dma_start(
                    w2T_s[:, :, hs], w2T_d[:, hs].rearrange("(o p) d -> p o d", p=P))

            ycontribs = [dram.tile([YC_ROWS, D], f32, tag=f"yc{q}", name=f"yc{q}")
                         for q in range(NQ)]
            yshards = [dram.tile([RSH, D], f32, tag=f"ys{q}", name=f"ys{q}")
                       for q in range(NQ)]
            wgtid_ds = [dram.tile([YC_ROWS, 2], f32, tag=f"wt{q}", name=f"wt{q}")
                        for q in range(NQ)]

            # --- zero-fill contribution buffers & list pads ---
            zt = wpool.tile([P, D], f32, tag="zt")
            nc.vector.memset(zt[:], 0.0)
            for q in range(NQ):
                for r in range(YC_ROWS // P):
                    nc.sync.dma_start(ycontribs[q][r * P:(r + 1) * P, :], zt[:])
                ft = cpool.tile([P, CT, 2], f32, tag="ft")
                nc.vector.memset(ft[:], 0.0)
                nc.vector.memset(ft[:, :, 1:2], float((q + 1) * RT))
                nc.sync.dma_start(
                    wgtid_ds[q][0:CAP, :].rearrange("(c p) w -> p c w", p=P), ft[:])

            # ============ per-range pipelines ============
            for q in range(NQ):
                # ---- gate over the range's 1024 tokens (true fp32) ----
                wgt_all = wapool.tile([P, E], f32, tag="wgtall")  # col = t_tile
                for half in range(2):
                    t0 = q * RT + half * TCH
                    xg_s = xgpool.tile([P, DK, TCH], f32, tag="xg")
                    nc.sync.dma_start(
                        xg_s[:],
                        xT_d[:, t0:t0 + TCH].rearrange("(o p) t -> p o t", p=P))
                    for tt in range(4):
                        f = half * 4 + tt
                        ps_g = psmall.tile([P, E], f32, tag="sm")
                        for dk in range(DK):
                            nc.tensor.matmul(
                                ps_g[:],
                                lhsT=xg_s[:, dk, tt * P:(tt + 1) * P],
                                rhs=gwT_s[:, dk, :],
                                start=(dk == 0), stop=(dk == DK - 1))
                        negmx = gpool.tile([P, 1], f32, tag="negmx")
                        nc.vector.tensor_reduce(
                            negmx[:], ps_g[:], mybir.AxisListType.X,
                            mybir.AluOpType.max)
                        nc.vector.tensor_scalar_mul(negmx[:], negmx[:], -1.0)
                        probs = gpool.tile([P, E], f32, tag="probs")
                        sumexp = gpool.tile([P, 1], f32, tag="sumexp")
                        nc.scalar.activation(
                            probs[:], ps_g[:], mybir.ActivationFunctionType.Exp,
                            bias=negmx[:, 0:1], accum_out=sumexp[:, 0:1])
                        recip = gpool.tile([P, 1], f32, tag="recip")
                        nc.vector.reciprocal(recip[:], sumexp[:])
                        nc.vector.tensor_scalar_mul(
                            probs[:], probs[:], recip[:, 0:1])
                        mx8 = gpool.tile([P, 8], f32, tag="mx8")
                        nc.vector.max(mx8[:], probs[:])
                        ge = gpool.tile([P, 1], f32, tag="ge")
                        nc.vector.tensor_tensor(
                            ge[:], probs[:, 0:1], mx8[:, 1:2],
                            mybir.AluOpType.is_ge)
                        nc.vector.tensor_mul(
                            wgt_all[:, f:f + 1], probs[:, 0:1], ge[:])

                # ---- compaction: list positions via matmul prefix sums ----
                m = cpool.tile([P, E], f32, tag="m")
                nc.vector.tensor_scalar(
                    m[:], wgt_all[:], 0.0, scalar2=None,
                    op0=mybir.AluOpType.is_gt)
                psA = psmall.tile([P, E], f32, tag="sm")
                nc.tensor.matmul(psA[:], lhsT=utri_s[:], rhs=m[:],
                                 start=True, stop=True)
                psC = psmall.tile([P, E], f32, tag="sm")
                nc.tensor.matmul(psC[:], lhsT=ones_s[:], rhs=m[:],
                                 start=True, stop=True)
                pos = cpool.tile([P, E], f32, tag="pos")
                nc.vector.tensor_copy(pos[:], psA[:])
                ctot = cpool.tile([P, E], f32, tag="ctot")
                nc.vector.tensor_copy(ctot[:], psC[:])
                # pos += exclusive cumsum over columns of ctot
                for f in range(1, E):
                    nc.vector.tensor_add(
                        ctot[:, f:f + 1], ctot[:, f:f + 1], ctot[:, f - 1:f])
                for f in range(1, E):
                    nc.vector.tensor_add(
                        pos[:, f:f + 1], pos[:, f:f + 1], ctot[:, f - 1:f])
                # unrouted tokens -> trash slot RT
                nc.vector.tensor_scalar_add(pos[:], pos[:], float(-RT))
                nc.vector.tensor_mul(pos[:], pos[:], m[:])
                nc.vector.tensor_scalar_add(pos[:], pos[:], float(RT))
                posi = cpool.tile([P, E], i32, tag="posi")
                nc.vector.tensor_copy(posi[:], pos[:])

                # scatter [wgt, tid] rows into the compacted list
                sw = cpool.tile([P, E, 2], f32, tag="sw")
                nc.vector.tensor_copy(sw[:, :, 0], wgt_all[:])
                nc.vector.tensor_scalar_add(
                    sw[:, :, 1], tidb_s[:], float(q * RT))
                for f in range(E):
                    nc.gpsimd.indirect_dma_start(
                        out=wgtid_ds[q][:, :],
                        out_offset=bass.IndirectOffsetOnAxis(
                            ap=posi[:, f:f + 1], axis=0),
                        in_=sw[:, f, :],
                        in_offset=None)

                # ---- read back compacted list, gather x rows ----
                wt_sb = cpool.tile([P, CT, 2], f32, tag="wt_sb")
                nc.sync.dma_start(
                    wt_sb[:],
                    wgtid_ds[q][0:CAP, :].rearrange("(c p) w -> p c w", p=P))
                gidx = cpool.tile([P, CT], i32, tag="gidx")
                nc.vector.tensor_copy(gidx[:], wt_sb[:, :, 1])
                yidx = cpool.tile([P, CT], f32, tag="yidxf")
                nc.vector.tensor_scalar_add(
                    yidx[:], wt_sb[:, :, 1], float(-q * RT))
                yidxi = cpool.tile([P, CT], i32, tag="yidxi")
                nc.vector.tensor_copy(yidxi[:], yidx[:])

                xeT = xtpool.tile([P, DK, CAP], f32r, tag="xeT")
                for ct in range(CT):
                    xe = xepool.tile([P, D], f32r, tag="xe")
                    nc.gpsimd.indirect_dma_start(
                        out=xe[:],
                        out_offset=None,
                        in_=x_d[:, :],
                        in_offset=bass.IndirectOffsetOnAxis(
                            ap=gidx[:, ct:ct + 1], axis=0))
                    for dk in range(DK):
                        ptr = psmall.tile([P, P], f32r, tag="sm")
                        nc.tensor.transpose(
                            ptr[:], xe[:, dk * P:(dk + 1) * P], ident_s[:])
                        nc.vector.tensor_copy(
                            xeT[:, dk, ct * P:(ct + 1) * P], ptr[:])

                # ---- SwiGLU on the compacted tokens ----
                aT = apool.tile([P, IK, CAP], f32r, tag="aT")
                for ik in range(IK):
                    isl = slice(ik * P, (ik + 1) * P)
                    ph = psum.tile([P, CAP], f32, tag="ph")
                    for dk in range(DK):
                        nc.tensor.matmul(
                            ph[:], lhsT=w1T_s[:, dk, isl], rhs=xeT[:, dk, :],
                            start=(dk == 0), stop=(dk == DK - 1))
                    pg = psum.tile([P, CAP], f32, tag="pg")
                    for dk in range(DK):
                        nc.tensor.matmul(
                            pg[:], lhsT=w3T_s[:, dk, isl], rhs=xeT[:, dk, :],
                            start=(dk == 0), stop=(dk == DK - 1))
                    sil = spool.tile([P, CAP], f32r, tag="sil")
                    nc.scalar.activation(
                        sil[:], ph[:], mybir.ActivationFunctionType.Silu)
                    nc.vector.tensor_mul(aT[:, ik, :], sil[:], pg[:])

                # ---- y rows = (a^T)^T @ w2^T, scaled, scattered back ----
                for ct in range(CT):
                    yt = ypool.tile([P, D], f32, tag="yt")
                    for dc in range(2):
                        py = pyps.tile([P, TCH], f32, tag="py")
                        for ik in range(IK):
                            nc.tensor.matmul(
                                py[:],
                                lhsT=aT[:, ik, ct * P:(ct + 1) * P],
                                rhs=w2T_s[:, ik, dc * TCH:(dc + 1) * TCH],
                                start=(ik == 0), stop=(ik == IK - 1))
                        nc.vector.tensor_scalar_mul(
                            yt[:, dc * TCH:(dc + 1) * TCH], py[:],
                            wt_sb[:, ct, 0:1])
                    nc.gpsimd.indirect_dma_start(
                        out=ycontribs[q][:, :],
                        out_offset=bass.IndirectOffsetOnAxis(
                            ap=yidxi[:, ct:ct + 1], axis=0),
                        in_=yt[:],
                        in_offset=None)

                # ---- combine this range across cores ----
                nc.gpsimd.collective_compute(
                    "ReduceScatter",
                    mybir.AluOpType.add,
                    replica_groups=[list(range(NCORES))],
                    ins=[ycontribs[q][0:RT, :]],
                    outs=[yshards[q].opt()],
                )
                nc.scalar.dma_start(y_d[q * RSH:(q + 1) * RSH, :], yshards[q][:])
    nc.compile()
    return nc


def _get_nc():
    global _CACHED_NC
    if _CACHED_NC is None:
        _CACHED_NC = _build()
    return _CACHED_NC


def _in_maps(x, gate_w, w1, w3, w2):
    x = np.asarray(x, dtype=np.float32)
    gate_w = np.asarray(gate_w, dtype=np.float32)
    xT = np.ascontiguousarray(x.T)
    xpad = np.zeros((XPAD_ROWS, D), dtype=np.float32)
    xpad[:T] = x

    # host-side capacity check against the actual gate (cheap, exact)
    s = x @ gate_w.T
    thr = np.sort(s, axis=1)[:, -TOPK]          # 2nd-largest score
    routed = s >= thr[:, None]                  # [T, E]
    cnt = routed.reshape(NQ, RT, E).sum(axis=1)  # [NQ, E]
    if cnt.max() > CAP:
        raise RuntimeError(f"routing capacity exceeded: {cnt.max()} > {CAP}")

    utri = np.triu(np.ones((P, P), np.float32), k=1)
    ones = np.ones((P, P), np.float32)
    ident = np.eye(P, dtype=np.float32)
    tidb = (np.arange(E)[None, :] * P + np.arange(P)[:, None]).astype(np.float32)

    maps = []
    for e in range(NCORES):
        perm = [e] + [j for j in range(E) if j != e]
        gwT = np.ascontiguousarray(gate_w[perm].T)
        maps.append({
            "xT": xT,
            "x": xpad,
            "gwT": gwT,
            "w1T": np.ascontiguousarray(np.asarray(w1[e], np.float32).T),
            "w3T": np.ascontiguousarray(np.asarray(w3[e], np.float32).T),
            "w2T": np.ascontiguousarray(np.asarray(w2[e], np.float32).T),
            "utri": utri,
            "ones": ones,
            "ident": ident,
            "tidb": tidb,
        })
    return maps


def run(x, gate_w, w1, w3, w2, trace=False, trace_cores=None):
    nc = _get_nc()
    maps = _in_maps(x, gate_w, w1, w3, w2)
    res = run_bass_kernel_spmd(
        nc, maps, core_ids=list(range(NCORES)), trace=trace,
        trace_cores=trace_cores)
    # core r's output block q (128 rows) holds tokens [1024q + 128r, +128)
    y = np.empty((T, D), dtype=np.float32)
    for r in range(NCORES):
        yr = res.results[r]["y"]
        for q in range(NQ):
            t0 = q * RT + r * RSH
            y[t0:t0 + RSH] = yr[q * RSH:(q + 1) * RSH]
    return y, res


def kernel(x, gate_w, w1, w3, w2):
    y, _ = run(x, gate_w, w1, w3, w2, trace=False)
    return y.astype(np.float32)


# revision 10
# speedup vs baseline: 2.1472x; 2.1472x over previous
"""MoE SwiGLU (T=4096, D=I=1024, E=8, top-2) on 8 Trainium2 NeuronCores.

Expert-parallel with on-device routing: core e holds expert e's weights
in SBUF.  The gate (scores -> softmax -> top-2) is replicated on every
core in true fp32.  Each core then COMPACTS the token ids routed to its
expert (matmul prefix-sums + indirect scatter), gathers just those x
rows (indirect DMA), computes SwiGLU only for them (float32r matmuls at
full PE rate), scales by the routing weight, and scatters the rows into
a zeroed per-range contribution buffer.  Four token-range ReduceScatters
overlap compute; the host reassembles the 8 shards.

Work is organized in 4 token ranges of 1024; per (core, range) the
routed token count is ~256 +- 14 (capacity 384, overflow checked on the
host against the actual gate before launch).
"""
import os
import sys

import numpy as np

for _p in ("/opt/trn_rl_repo", "/root/.axon_site/_ro/trn_rl_repo"):
    if os.path.isdir(_p) and _p not in sys.path:
        sys.path.append(_p)

import concourse.bass as bass  # noqa: E402
import concourse.mybir as mybir  # noqa: E402
import concourse.tile as tile  # noqa: E402
from concourse import bacc  # noqa: E402
from concourse.bass_utils import run_bass_kernel_spmd  # noqa: E402

P = 128
T, D, I, E, TOPK = 4096, 1024, 1024, 8, 2
NCORES = 8
TCH = 512            # gate token chunk (matmul free dim)
NCH = T // TCH       # 8
DK = D // P          # 8
IK = I // P          # 8
NQ = 4               # ReduceScatter ranges
RT = T // NQ         # 1024 tokens per range
RSH = RT // NCORES   # 128-token shard per core per range
CAP = 384            # routed-token capacity per (core, range)
CT = CAP // P        # 3 c-tiles per range
YC_ROWS = RT + P     # contribution rows + trash row region
XPAD_ROWS = T + P    # x padded with zero rows (gather trash target)
f32 = mybir.dt.float32
f32r = mybir.dt.float32r
i32 = mybir.dt.int32

_CACHED_NC = None


def _build():
    nc = bacc.Bacc("TRN2", target_bir_lowering=False, debug=False,
                   num_devices=NCORES)
    xT_d = nc.dram_tensor("xT", [D, T], f32, kind="ExternalInput")
    x_d = nc.dram_tensor("x", [XPAD_ROWS, D], f32r, kind="ExternalInput")
    gwT_d = nc.dram_tensor("gwT", [D, E], f32, kind="ExternalInput")
    w1T_d = nc.dram_tensor("w1T", [D, I], f32r, kind="ExternalInput")
    w3T_d = nc.dram_tensor("w3T", [D, I], f32r, kind="ExternalInput")
    w2T_d = nc.dram_tensor("w2T", [I, D], f32r, kind="ExternalInput")
    utri_d = nc.dram_tensor("utri", [P, P], f32, kind="ExternalInput")
    ones_d = nc.dram_tensor("ones", [P, P], f32, kind="ExternalInput")
    ident_d = nc.dram_tensor("ident", [P, P], f32r, kind="ExternalInput")
    tidb_d = nc.dram_tensor("tidb", [P, E], f32, kind="ExternalInput")
    y_d = nc.dram_tensor("y", [NQ * RSH, D], f32, kind="ExternalOutput")

    with tile.TileContext(nc) as tc:
        with tc.tile_pool(name="wpool", bufs=1) as wpool, \
             tc.tile_pool(name="xgpool", bufs=2) as xgpool, \
             tc.tile_pool(name="gpool", bufs=2) as gpool, \
             tc.tile_pool(name="wapool", bufs=5) as wapool, \
             tc.tile_pool(name="cpool", bufs=5) as cpool, \
             tc.tile_pool(name="xepool", bufs=3) as xepool, \
             tc.tile_pool(name="xtpool", bufs=1) as xtpool, \
             tc.tile_pool(name="apool", bufs=1) as apool, \
             tc.tile_pool(name="spool", bufs=2) as spool, \
             tc.tile_pool(name="ypool", bufs=2) as ypool, \
             tc.tile_pool(name="psum", bufs=2, space="PSUM") as psum, \
             tc.tile_pool(name="pyps", bufs=2, space="PSUM") as pyps, \
             tc.tile_pool(name="psmall", bufs=2, space="PSUM") as psmall, \
             tc.tile_pool(name="dram", bufs=1, space="DRAM") as dram:

            # --- constants + resident weights ---
            gwT_s = wpool.tile([P, DK, E], f32, tag="gw")
            nc.sync.dma_start(gwT_s[:], gwT_d[:, :].rearrange("(o p) e -> p o e", p=P))
            utri_s = wpool.tile([P, P], f32, tag="utri")
            nc.sync.dma_start(utri_s[:], utri_d[:, :])
            ones_s = wpool.tile([P, P], f32, tag="ones")
            nc.sync.dma_start(ones_s[:], ones_d[:, :])
            ident_s = wpool.tile([P, P], f32r, tag="ident")
            nc.sync.dma_start(ident_s[:], ident_d[:, :])
            tidb_s = wpool.tile([P, E], f32, tag="tidb")
            nc.sync.dma_start(tidb_s[:], tidb_d[:, :])

            w1T_s = wpool.tile([P, DK, I], f32r, tag="w1")
            w3T_s = wpool.tile([P, DK, I], f32r, tag="w3")
            w2T_s = wpool.tile([P, IK, D], f32r, tag="w2")
            for h in range(4):
                hs = slice(h * (I // 4), (h + 1) * (I // 4))
                nc.sync.dma_start(
                    w1T_s[:, :, hs], w1T_d[:, hs].rearrange("(o p) i -> p o i", p=P))
                nc.sync.dma_start(
                    w3T_s[:, :, hs], w3T_d[:, hs].rearrange("(o p) i -> p o i", p=P))
                nc.sync.dma_start(
                    w2T_s[:, :, hs], w2T_d[:, hs].rearrange("(o p) d -> p o d", p=P))

            ycontribs = [dram.tile([YC_ROWS, D], f32, tag=f"yc{q}", name=f"yc{q}")
                         for q in range(NQ)]
            yshards = [dram.tile([RSH, D], f32, tag=f"ys{q}", name=f"ys{q}")
                       for q in range(NQ)]
            wgtid_ds = [dram.tile([YC_ROWS, 2], f32, tag=f"wt{q}", name=f"wt{q}")
                        for q in range(NQ)]

            # --- zero-fill contribution buffers & list pads (scalar queue:
            #     idle early, keeps sync free for input streaming) ---
            zt = wpool.tile([P, D], f32, tag="zt")
            nc.vector.memset(zt[:], 0.0)
            for q in range(NQ):
                for r in range(YC_ROWS // P):
                    nc.scalar.dma_start(ycontribs[q][r * P:(r + 1) * P, :], zt[:])
                ft = cpool.tile([P, CT, 2], f32, tag="ft", name=f"ft{q}")
                nc.vector.memset(ft[:], 0.0)
                nc.vector.memset(ft[:, :, 1:2], float((q + 1) * RT))
                nc.scalar.dma_start(
                    wgtid_ds[q][0:CAP, :].rearrange("(c p) w -> p c w", p=P), ft[:])

            # ============ phase A: gate for all ranges (true fp32) ============
            wgt_alls = []
            for q in range(NQ):
                wgt_all = wapool.tile([P, E], f32, tag="wgtall", name=f"wa{q}")
                wgt_alls.append(wgt_all)
                for half in range(2):
                    t0 = q * RT + half * TCH
                    xg_s = xgpool.tile([P, DK, TCH], f32, tag="xg")
                    nc.sync.dma_start(
                        xg_s[:],
                        xT_d[:, t0:t0 + TCH].rearrange("(o p) t -> p o t", p=P))
                    for tt in range(4):
                        f = half * 4 + tt
                        ps_g = psmall.tile([P, E], f32, tag="sm")
                        for dk in range(DK):
                            nc.tensor.matmul(
                                ps_g[:],
                                lhsT=xg_s[:, dk, tt * P:(tt + 1) * P],
                                rhs=gwT_s[:, dk, :],
                                start=(dk == 0), stop=(dk == DK - 1))
                        negmx = gpool.tile([P, 1], f32, tag="negmx")
                        nc.vector.tensor_reduce(
                            negmx[:], ps_g[:], mybir.AxisListType.X,
                            mybir.AluOpType.max)
                        nc.vector.tensor_scalar_mul(negmx[:], negmx[:], -1.0)
                        probs = gpool.tile([P, E], f32, tag="probs")
                        sumexp = gpool.tile([P, 1], f32, tag="sumexp")
                        nc.scalar.activation(
                            probs[:], ps_g[:], mybir.ActivationFunctionType.Exp,
                            bias=negmx[:, 0:1], accum_out=sumexp[:, 0:1])
                        recip = gpool.tile([P, 1], f32, tag="recip")
                        nc.vector.reciprocal(recip[:], sumexp[:])
                        nc.vector.tensor_scalar_mul(
                            probs[:], probs[:], recip[:, 0:1])
                        mx8 = gpool.tile([P, 8], f32, tag="mx8")
                        nc.vector.max(mx8[:], probs[:])
                        ge = gpool.tile([P, 1], f32, tag="ge")
                        nc.vector.tensor_tensor(
                            ge[:], probs[:, 0:1], mx8[:, 1:2],
                            mybir.AluOpType.is_ge)
                        nc.vector.tensor_mul(
                            wgt_all[:, f:f + 1], probs[:, 0:1], ge[:])

            # ===== phase B: compaction (prefix sums, list scatter, readback) ====
            wt_sbs, gidxs, yidxis = [], [], []
            for q in range(NQ):
                wgt_all = wgt_alls[q]
                m = cpool.tile([P, E], f32, tag="m", name=f"m{q}")
                nc.vector.tensor_scalar(
                    m[:], wgt_all[:], 0.0, scalar2=None,
                    op0=mybir.AluOpType.is_gt)
                psA = psmall.tile([P, E], f32, tag="sm")
                nc.tensor.matmul(psA[:], lhsT=utri_s[:], rhs=m[:],
                                 start=True, stop=True)
                psC = psmall.tile([P, E], f32, tag="sm")
                nc.tensor.matmul(psC[:], lhsT=ones_s[:], rhs=m[:],
                                 start=True, stop=True)
                pos = cpool.tile([P, E], f32, tag="pos", name=f"pos{q}")
                nc.vector.tensor_copy(pos[:], psA[:])
                ctot = cpool.tile([P, E], f32, tag="ctot", name=f"ct{q}")
                nc.vector.tensor_copy(ctot[:], psC[:])
                for f in range(1, E):
                    nc.vector.tensor_add(
                        ctot[:, f:f + 1], ctot[:, f:f + 1], ctot[:, f - 1:f])
                for f in range(1, E):
                    nc.vector.tensor_add(
                        pos[:, f:f + 1], pos[:, f:f + 1], ctot[:, f - 1:f])
                nc.vector.tensor_scalar_add(pos[:], pos[:], float(-RT))
                nc.vector.tensor_mul(pos[:], pos[:], m[:])
                nc.vector.tensor_scalar_add(pos[:], pos[:], float(RT))
                posi = cpool.tile([P, E], i32, tag="posi", name=f"pi{q}")
                nc.vector.tensor_copy(posi[:], pos[:])

                sw = cpool.tile([P, E, 2], f32, tag="sw", name=f"sw{q}")
                nc.vector.tensor_copy(sw[:, :, 0], wgt_all[:])
                nc.vector.tensor_scalar_add(
                    sw[:, :, 1], tidb_s[:], float(q * RT))
                for f in range(E):
                    nc.gpsimd.indirect_dma_start(
                        out=wgtid_ds[q][:, :],
                        out_offset=bass.IndirectOffsetOnAxis(
                            ap=posi[:, f:f + 1], axis=0),
                        in_=sw[:, f, :],
                        in_offset=None)

                wt_sb = cpool.tile([P, CT, 2], f32, tag="wt_sb", name=f"wsb{q}")
                nc.sync.dma_start(
                    wt_sb[:],
                    wgtid_ds[q][0:CAP, :].rearrange("(c p) w -> p c w", p=P))
                gidx = cpool.tile([P, CT], i32, tag="gidx", name=f"gi{q}")
                nc.vector.tensor_copy(gidx[:], wt_sb[:, :, 1])
                yidx = cpool.tile([P, CT], f32, tag="yidxf", name=f"yf{q}")
                nc.vector.tensor_scalar_add(
                    yidx[:], wt_sb[:, :, 1], float(-q * RT))
                yidxi = cpool.tile([P, CT], i32, tag="yidxi", name=f"yi{q}")
                nc.vector.tensor_copy(yidxi[:], yidx[:])
                wt_sbs.append(wt_sb); gidxs.append(gidx); yidxis.append(yidxi)

            # ============ phase C: per-range gather/compute/combine ============
            for q in range(NQ):
                wt_sb, gidx, yidxi = wt_sbs[q], gidxs[q], yidxis[q]
                xeT = xtpool.tile([P, DK, CAP], f32r, tag="xeT")
                for ct in range(CT):
                    xe = xepool.tile([P, D], f32r, tag="xe")
                    nc.gpsimd.indirect_dma_start(
                        out=xe[:],
                        out_offset=None,
                        in_=x_d[:, :],
                        in_offset=bass.IndirectOffsetOnAxis(
                            ap=gidx[:, ct:ct + 1], axis=0))
                    for dk in range(DK):
                        ptr = psmall.tile([P, P], f32r, tag="sm")
                        nc.tensor.transpose(
                            ptr[:], xe[:, dk * P:(dk + 1) * P], ident_s[:])
                        nc.vector.tensor_copy(
                            xeT[:, dk, ct * P:(ct + 1) * P], ptr[:])

                aT = apool.tile([P, IK, CAP], f32r, tag="aT")
                for ik in range(IK):
                    isl = slice(ik * P, (ik + 1) * P)
                    ph = psum.tile([P, CAP], f32, tag="ph")
                    for dk in range(DK):
                        nc.tensor.matmul(
                            ph[:], lhsT=w1T_s[:, dk, isl], rhs=xeT[:, dk, :],
                            start=(dk == 0), stop=(dk == DK - 1))
                    pg = psum.tile([P, CAP], f32, tag="pg")
                    for dk in range(DK):
                        nc.tensor.matmul(
                            pg[:], lhsT=w3T_s[:, dk, isl], rhs=xeT[:, dk, :],
                            start=(dk == 0), stop=(dk == DK - 1))
                    sil = spool.tile([P, CAP], f32r, tag="sil")
                    nc.scalar.activation(
                        sil[:], ph[:], mybir.ActivationFunctionType.Silu)
                    nc.vector.tensor_mul(aT[:, ik, :], sil[:], pg[:])

                for ct in range(CT):
                    yt = ypool.tile([P, D], f32, tag="yt")
                    for dc in range(2):
                        py = pyps.tile([P, TCH], f32, tag="py")
                        for ik in range(IK):
                            nc.tensor.matmul(
                                py[:],
                                lhsT=aT[:, ik, ct * P:(ct + 1) * P],
                                rhs=w2T_s[:, ik, dc * TCH:(dc + 1) * TCH],
                                start=(ik == 0), stop=(ik == IK - 1))
                        nc.vector.tensor_scalar_mul(
                            yt[:, dc * TCH:(dc + 1) * TCH], py[:],
                            wt_sb[:, ct, 0:1])
                    nc.gpsimd.indirect_dma_start(
                        out=ycontribs[q][:, :],
                        out_offset=bass.IndirectOffsetOnAxis(
                            ap=yidxi[:, ct:ct + 1], axis=0),
                        in_=yt[:],
                        in_offset=None)

                nc.gpsimd.collective_compute(
                    "ReduceScatter",
                    mybir.AluOpType.add,
                    replica_groups=[list(range(NCORES))],
                    ins=[ycontribs[q][0:RT, :].opt()],
                    outs=[yshards[q].opt()],
                )

            # ============ phase D: ship shards to the output ============
            for q in range(NQ):
                nc.sync.dma_start(y_d[q * RSH:(q + 1) * RSH, :], yshards[q][:])
    nc.compile()
    return nc


def _get_nc():
    global _CACHED_NC
    if _CACHED_NC is None:
        _CACHED_NC = _build()
    return _CACHED_NC


def _in_maps(x, gate_w, w1, w3, w2):
    x = np.asarray(x, dtype=np.float32)
    gate_w = np.asarray(gate_w, dtype=np.float32)
    xT = np.ascontiguousarray(x.T)
    xpad = np.zeros((XPAD_ROWS, D), dtype=np.float32)
    xpad[:T] = x

    # host-side capacity check against the actual gate (cheap, exact)
    s = x @ gate_w.T
    thr = np.sort(s, axis=1)[:, -TOPK]          # 2nd-largest score
    routed = s >= thr[:, None]                  # [T, E]
    cnt = routed.reshape(NQ, RT, E).sum(axis=1)  # [NQ, E]
    if cnt.max() > CAP:
        raise RuntimeError(f"routing capacity exceeded: {cnt.max()} > {CAP}")

    utri = np.triu(np.ones((P, P), np.float32), k=1)
    ones = np.ones((P, P), np.float32)
    ident = np.eye(P, dtype=np.float32)
    tidb = (np.arange(E)[None, :] * P + np.arange(P)[:, None]).astype(np.float32)

    maps = []
    for e in range(NCORES):
        perm = [e] + [j for j in range(E) if j != e]
        gwT = np.ascontiguousarray(gate_w[perm].T)
        maps.append({
            "xT": xT,
            "x": xpad,
            "gwT": gwT,
            "w1T": np.ascontiguousarray(np.asarray(w1[e], np.float32).T),
            "w3T": np.ascontiguousarray(np.asarray(w3[e], np.float32).T),
            "w2T": np.ascontiguousarray(np.asarray(w2[e], np.float32).T),
            "utri": utri,
            "ones": ones,
            "ident": ident,
            "tidb": tidb,
        })
    return maps


def run(x, gate_w, w1, w3, w2, trace=False, trace_cores=None):
    nc = _get_nc()
    maps = _in_maps(x, gate_w, w1, w3, w2)
    res = run_bass_kernel_spmd(
        nc, maps, core_ids=list(range(NCORES)), trace=trace,
        trace_cores=trace_cores)
    # core r's output block q (128 rows) holds tokens [1024q + 128r, +128)
    y = np.empty((T, D), dtype=np.float32)
    for r in range(NCORES):
        yr = res.results[r]["y"]
        for q in range(NQ):
            t0 = q * RT + r * RSH
            y[t0:t0 + RSH] = yr[q * RSH:(q + 1) * RSH]
    return y, res


def kernel(x, gate_w, w1, w3, w2):
    y, _ = run(x, gate_w, w1, w3, w2, trace=False)
    return y.astype(np.float32)


# revision 12
# speedup vs baseline: 2.9884x; 1.3918x over previous
"""MoE SwiGLU (T=4096, D=I=1024, E=8, top-2) on 8 Trainium2 NeuronCores.

Expert-parallel with on-device routing: core e holds expert e's weights
in SBUF.  The gate (scores -> softmax -> top-2) is replicated on every
core in true fp32.  Each core then COMPACTS the token ids routed to its
expert (matmul prefix-sums + indirect scatter), gathers just those x
rows (indirect DMA), computes SwiGLU only for them (float32r matmuls at
full PE rate), scales by the routing weight, and scatters the rows into
a zeroed per-range contribution buffer.  Four token-range ReduceScatters
overlap compute; the host reassembles the 8 shards.

Work is organized in 4 token ranges of 1024; per (core, range) the
routed token count is ~256 +- 14 (capacity 384, overflow checked on the
host against the actual gate before launch).
"""
import os
import sys

import numpy as np

for _p in ("/opt/trn_rl_repo", "/root/.axon_site/_ro/trn_rl_repo"):
    if os.path.isdir(_p) and _p not in sys.path:
        sys.path.append(_p)

import concourse.bass as bass  # noqa: E402
import concourse.mybir as mybir  # noqa: E402
import concourse.tile as tile  # noqa: E402
from concourse import bacc  # noqa: E402
from concourse.bass_utils import run_bass_kernel_spmd  # noqa: E402

P = 128
T, D, I, E, TOPK = 4096, 1024, 1024, 8, 2
NCORES = 8
TCH = 512            # gate token chunk (matmul free dim)
NCH = T // TCH       # 8
DK = D // P          # 8
IK = I // P          # 8
NQ = 4               # ReduceScatter ranges
RT = T // NQ         # 1024 tokens per range
RSH = RT // NCORES   # 128-token shard per core per range
CAP = 384            # routed-token capacity per (core, range)
CT = CAP // P        # 3 c-tiles per range
YC_ROWS = RT + P     # contribution rows + trash row region
XPAD_ROWS = T + P    # x padded with zero rows (gather trash target)
f32 = mybir.dt.float32
f32r = mybir.dt.float32r
i32 = mybir.dt.int32

_CACHED_NC = None


def _build():
    nc = bacc.Bacc("TRN2", target_bir_lowering=False, debug=False,
                   num_devices=NCORES)
    xT_d = nc.dram_tensor("xT", [D, T], f32, kind="ExternalInput")
    x_d = nc.dram_tensor("x", [XPAD_ROWS, D], f32r, kind="ExternalInput")
    gwT_d = nc.dram_tensor("gwT", [D, E], f32, kind="ExternalInput")
    w1T_d = nc.dram_tensor("w1T", [D, I], f32r, kind="ExternalInput")
    w3T_d = nc.dram_tensor("w3T", [D, I], f32r, kind="ExternalInput")
    w2T_d = nc.dram_tensor("w2T", [I, D], f32r, kind="ExternalInput")
    utri_d = nc.dram_tensor("utri", [P, P], f32, kind="ExternalInput")
    ones_d = nc.dram_tensor("ones", [P, P], f32, kind="ExternalInput")
    ident_d = nc.dram_tensor("ident", [P, P], f32r, kind="ExternalInput")
    tidb_d = nc.dram_tensor("tidb", [P, E], f32, kind="ExternalInput")
    sr_d = nc.dram_tensor("sr", [P, CT * P], f32, kind="ExternalInput")
    y_d = nc.dram_tensor("y", [NQ * RSH, D], f32, kind="ExternalOutput")

    with tile.TileContext(nc) as tc:
        with tc.tile_pool(name="wpool", bufs=1) as wpool, \
             tc.tile_pool(name="xgpool", bufs=2) as xgpool, \
             tc.tile_pool(name="gpool", bufs=2) as gpool, \
             tc.tile_pool(name="wapool", bufs=5) as wapool, \
             tc.tile_pool(name="cpool", bufs=5) as cpool, \
             tc.tile_pool(name="xepool", bufs=3) as xepool, \
             tc.tile_pool(name="xtpool", bufs=1) as xtpool, \
             tc.tile_pool(name="apool", bufs=1) as apool, \
             tc.tile_pool(name="spool", bufs=2) as spool, \
             tc.tile_pool(name="ypool", bufs=2) as ypool, \
             tc.tile_pool(name="psum", bufs=2, space="PSUM") as psum, \
             tc.tile_pool(name="pyps", bufs=2, space="PSUM") as pyps, \
             tc.tile_pool(name="psmall", bufs=2, space="PSUM") as psmall, \
             tc.tile_pool(name="dram", bufs=1, space="DRAM") as dram:

            # --- constants + resident weights ---
            gwT_s = wpool.tile([P, DK, E], f32, tag="gw")
            nc.sync.dma_start(gwT_s[:], gwT_d[:, :].rearrange("(o p) e -> p o e", p=P))
            utri_s = wpool.tile([P, P], f32, tag="utri")
            nc.sync.dma_start(utri_s[:], utri_d[:, :])
            ones_s = wpool.tile([P, P], f32, tag="ones")
            nc.sync.dma_start(ones_s[:], ones_d[:, :])
            ident_s = wpool.tile([P, P], f32r, tag="ident")
            nc.sync.dma_start(ident_s[:], ident_d[:, :])
            tidb_s = wpool.tile([P, E], f32, tag="tidb")
            nc.sync.dma_start(tidb_s[:], tidb_d[:, :])
            sr_s = wpool.tile([P, CT * P], f32, tag="sr")
            nc.sync.dma_start(sr_s[:], sr_d[:, :])
            identf_s = wpool.tile([P, P], f32, tag="identf")
            nc.vector.tensor_copy(identf_s[:], ident_s[:])

            w1T_s = wpool.tile([P, DK, I], f32r, tag="w1")
            w3T_s = wpool.tile([P, DK, I], f32r, tag="w3")
            w2T_s = wpool.tile([P, IK, D], f32r, tag="w2")
            for h in range(4):
                hs = slice(h * (I // 4), (h + 1) * (I // 4))
                nc.sync.dma_start(
                    w1T_s[:, :, hs], w1T_d[:, hs].rearrange("(o p) i -> p o i", p=P))
                nc.sync.dma_start(
                    w3T_s[:, :, hs], w3T_d[:, hs].rearrange("(o p) i -> p o i", p=P))
                nc.sync.dma_start(
                    w2T_s[:, :, hs], w2T_d[:, hs].rearrange("(o p) d -> p o d", p=P))

            ycontribs = [dram.tile([YC_ROWS, D], f32, tag=f"yc{q}", name=f"yc{q}")
                         for q in range(NQ)]
            yshards = [dram.tile([RSH, D], f32, tag=f"ys{q}", name=f"ys{q}")
                       for q in range(NQ)]

            # --- zero-fill contribution buffers & list pads (scalar queue:
            #     idle early, keeps sync free for input streaming) ---
            zt = wpool.tile([P, D], f32, tag="zt")
            nc.vector.memset(zt[:], 0.0)
            for q in range(NQ):
                for r in range(YC_ROWS // P):
                    nc.scalar.dma_start(ycontribs[q][r * P:(r + 1) * P, :], zt[:])

            # ============ phase A: gate for all ranges (true fp32) ============
            # scores^T [E, tokens] with N=512 matmuls, PE-transposed back to
            # [tokens, E] tiles for the softmax/top-2.
            wgt_alls = []
            for q in range(NQ):
                wgt_all = wapool.tile([P, E], f32, tag="wgtall", name=f"wa{q}")
                wgt_alls.append(wgt_all)
                for half in range(2):
                    t0 = q * RT + half * TCH
                    xg_s = xgpool.tile([P, DK, TCH], f32, tag="xg")
                    nc.sync.dma_start(
                        xg_s[:],
                        xT_d[:, t0:t0 + TCH].rearrange("(o p) t -> p o t", p=P))
                    ps_sT = psmall.tile([E, TCH], f32, tag="sm")
                    for dk in range(DK):
                        nc.tensor.matmul(
                            ps_sT[:], lhsT=gwT_s[:, dk, :], rhs=xg_s[:, dk, :],
                            start=(dk == 0), stop=(dk == DK - 1))
                    sT_sb = gpool.tile([E, TCH], f32, tag="sTsb")
                    nc.vector.tensor_copy(sT_sb[:], ps_sT[:])
                    for tt in range(4):
                        f = half * 4 + tt
                        ps_g = psmall.tile([P, E], f32, tag="sm")
                        nc.tensor.transpose(
                            ps_g[:], sT_sb[:, tt * P:(tt + 1) * P],
                            identf_s[:E, :E])
                        negmx = gpool.tile([P, 1], f32, tag="negmx")
                        nc.vector.tensor_reduce(
                            negmx[:], ps_g[:], mybir.AxisListType.X,
                            mybir.AluOpType.max)
                        nc.vector.tensor_scalar_mul(negmx[:], negmx[:], -1.0)
                        probs = gpool.tile([P, E], f32, tag="probs")
                        sumexp = gpool.tile([P, 1], f32, tag="sumexp")
                        nc.scalar.activation(
                            probs[:], ps_g[:], mybir.ActivationFunctionType.Exp,
                            bias=negmx[:, 0:1], accum_out=sumexp[:, 0:1])
                        recip = gpool.tile([P, 1], f32, tag="recip")
                        nc.vector.reciprocal(recip[:], sumexp[:])
                        nc.vector.tensor_scalar_mul(
                            probs[:], probs[:], recip[:, 0:1])
                        mx8 = gpool.tile([P, 8], f32, tag="mx8")
                        nc.vector.max(mx8[:], probs[:])
                        ge = gpool.tile([P, 1], f32, tag="ge")
                        nc.vector.tensor_tensor(
                            ge[:], probs[:, 0:1], mx8[:, 1:2],
                            mybir.AluOpType.is_ge)
                        nc.vector.tensor_mul(
                            wgt_all[:, f:f + 1], probs[:, 0:1], ge[:])

            # ===== phase B: compaction via prefix sums + one-hot matmuls =====
            # For each list slot s: gather-index/weight/occupancy recovered as
            # sum_t [pos[t]==s] * (tid, wgt, 1)[t]  -- no DRAM round trip.
            lists = []
            for q in range(NQ):
                wgt_all = wgt_alls[q]
                m = cpool.tile([P, E], f32, tag="m", name=f"m{q}")
                nc.vector.tensor_scalar(
                    m[:], wgt_all[:], 0.0, scalar2=None,
                    op0=mybir.AluOpType.is_gt)
                psA = psmall.tile([P, E], f32, tag="sm")
                nc.tensor.matmul(psA[:], lhsT=utri_s[:], rhs=m[:],
                                 start=True, stop=True)
                psC = psmall.tile([P, E], f32, tag="sm")
                nc.tensor.matmul(psC[:], lhsT=ones_s[:], rhs=m[:],
                                 start=True, stop=True)
                pos = cpool.tile([P, E], f32, tag="pos", name=f"pos{q}")
                nc.vector.tensor_copy(pos[:], psA[:])
                ctot = cpool.tile([P, E], f32, tag="ctot", name=f"ct{q}")
                nc.vector.tensor_copy(ctot[:], psC[:])
                for f in range(1, E):
                    nc.vector.tensor_add(
                        ctot[:, f:f + 1], ctot[:, f:f + 1], ctot[:, f - 1:f])
                for f in range(1, E):
                    nc.vector.tensor_add(
                        pos[:, f:f + 1], pos[:, f:f + 1], ctot[:, f - 1:f])
                nc.vector.tensor_scalar_add(pos[:], pos[:], float(-RT))
                nc.vector.tensor_mul(pos[:], pos[:], m[:])
                nc.vector.tensor_scalar_add(pos[:], pos[:], float(RT))

                # rhs payload per token: [tid, wgt, mask]
                pay = cpool.tile([P, E, 3], f32, tag="pay", name=f"pay{q}")
                nc.vector.tensor_scalar_add(
                    pay[:, :, 0], tidb_s[:], float(q * RT))
                nc.vector.tensor_copy(pay[:, :, 1], wgt_all[:])
                nc.vector.tensor_copy(pay[:, :, 2], m[:])

                lst = cpool.tile([P, CT, 3], f32, tag="lst", name=f"lst{q}")
                for ct in range(CT):
                    ps_l = psmall.tile([P, 3], f32, tag="sm")
                    for f in range(E):
                        ind = cpool.tile([P, P], f32, tag="ind")
                        nc.vector.tensor_tensor(
                            ind[:], pos[:, f:f + 1].to_broadcast([P, P]),
                            sr_s[:, ct * P:(ct + 1) * P],
                            mybir.AluOpType.is_equal)
                        nc.tensor.matmul(
                            ps_l[:], lhsT=ind[:], rhs=pay[:, f, :],
                            start=(f == 0), stop=(f == E - 1))
                    nc.vector.tensor_copy(lst[:, ct, :], ps_l[:])

                # pads (occ=0): gather trash x row, scatter to trash y row
                gidxf = cpool.tile([P, CT], f32, tag="gxf", name=f"gxf{q}")
                occ1 = cpool.tile([P, CT], f32, tag="occ1", name=f"occ1{q}")
                # gidx = tid + (1-occ)*T ; yidx = tid - q*RT + (1-occ)*(RT + q*RT)
                nc.vector.tensor_scalar(
                    occ1[:], lst[:, :, 2], -1.0, None,
                    op0=mybir.AluOpType.add)        # occ-1  (0 or -1)
                gidx_i = cpool.tile([P, CT], i32, tag="gidx", name=f"gi{q}")
                nc.vector.tensor_scalar(
                    gidxf[:], occ1[:], -float(T), None,
                    op0=mybir.AluOpType.mult)       # (1-occ)*T
                nc.vector.tensor_add(gidxf[:], gidxf[:], lst[:, :, 0])
                nc.vector.tensor_copy(gidx_i[:], gidxf[:])
                yidxf = cpool.tile([P, CT], f32, tag="yxf", name=f"yxf{q}")
                nc.vector.tensor_scalar(
                    yidxf[:], occ1[:], -float(RT + q * RT), None,
                    op0=mybir.AluOpType.mult)       # (1-occ)*(RT+q*RT)
                nc.vector.tensor_add(yidxf[:], yidxf[:], lst[:, :, 0])
                nc.vector.tensor_scalar_add(yidxf[:], yidxf[:], float(-q * RT))
                yidx_i = cpool.tile([P, CT], i32, tag="yidxi", name=f"yi{q}")
                nc.vector.tensor_copy(yidx_i[:], yidxf[:])
                lists.append((lst, gidx_i, yidx_i))

            # ============ phase C: per-range gather/compute/combine ============
            for q in range(NQ):
                lst, gidx, yidxi = lists[q]
                xeT = xtpool.tile([P, DK, CAP], f32r, tag="xeT")
                for ct in range(CT):
                    xe = xepool.tile([P, D], f32r, tag="xe")
                    nc.gpsimd.indirect_dma_start(
                        out=xe[:],
                        out_offset=None,
                        in_=x_d[:, :],
                        in_offset=bass.IndirectOffsetOnAxis(
                            ap=gidx[:, ct:ct + 1], axis=0))
                    for dk in range(DK):
                        ptr = psmall.tile([P, P], f32r, tag="sm")
                        nc.tensor.transpose(
                            ptr[:], xe[:, dk * P:(dk + 1) * P], ident_s[:])
                        nc.vector.tensor_copy(
                            xeT[:, dk, ct * P:(ct + 1) * P], ptr[:])

                aT = apool.tile([P, IK, CAP], f32r, tag="aT")
                for ik in range(IK):
                    isl = slice(ik * P, (ik + 1) * P)
                    ph = psum.tile([P, CAP], f32, tag="ph")
                    for dk in range(DK):
                        nc.tensor.matmul(
                            ph[:], lhsT=w1T_s[:, dk, isl], rhs=xeT[:, dk, :],
                            start=(dk == 0), stop=(dk == DK - 1))
                    pg = psum.tile([P, CAP], f32, tag="pg")
                    for dk in range(DK):
                        nc.tensor.matmul(
                            pg[:], lhsT=w3T_s[:, dk, isl], rhs=xeT[:, dk, :],
                            start=(dk == 0), stop=(dk == DK - 1))
                    sil = spool.tile([P, CAP], f32r, tag="sil")
                    nc.scalar.activation(
                        sil[:], ph[:], mybir.ActivationFunctionType.Silu)
                    nc.vector.tensor_mul(aT[:, ik, :], sil[:], pg[:])

                for ct in range(CT):
                    yt = ypool.tile([P, D], f32, tag="yt")
                    for dc in range(2):
                        py = pyps.tile([P, TCH], f32, tag="py")
                        for ik in range(IK):
                            nc.tensor.matmul(
                                py[:],
                                lhsT=aT[:, ik, ct * P:(ct + 1) * P],
                                rhs=w2T_s[:, ik, dc * TCH:(dc + 1) * TCH],
                                start=(ik == 0), stop=(ik == IK - 1))
                        nc.vector.tensor_scalar_mul(
                            yt[:, dc * TCH:(dc + 1) * TCH], py[:],
                            lst[:, ct, 1:2])
                    nc.gpsimd.indirect_dma_start(
                        out=ycontribs[q][:, :],
                        out_offset=bass.IndirectOffsetOnAxis(
                            ap=yidxi[:, ct:ct + 1], axis=0),
                        in_=yt[:],
                        in_offset=None)

                nc.gpsimd.collective_compute(
                    "ReduceScatter",
                    mybir.AluOpType.add,
                    replica_groups=[list(range(NCORES))],
                    ins=[ycontribs[q][0:RT, :].opt()],
                    outs=[yshards[q].opt()],
                )

            # ============ phase D: ship shards to the output ============
            for q in range(NQ):
                nc.sync.dma_start(y_d[q * RSH:(q + 1) * RSH, :], yshards[q][:])
    nc.compile()
    return nc


def _get_nc():
    global _CACHED_NC
    if _CACHED_NC is None:
        _CACHED_NC = _build()
    return _CACHED_NC


def _in_maps(x, gate_w, w1, w3, w2):
    x = np.asarray(x, dtype=np.float32)
    gate_w = np.asarray(gate_w, dtype=np.float32)
    xT = np.ascontiguousarray(x.T)
    xpad = np.zeros((XPAD_ROWS, D), dtype=np.float32)
    xpad[:T] = x

    # host-side capacity check against the actual gate (cheap, exact)
    s = x @ gate_w.T
    thr = np.sort(s, axis=1)[:, -TOPK]          # 2nd-largest score
    routed = s >= thr[:, None]                  # [T, E]
    cnt = routed.reshape(NQ, RT, E).sum(axis=1)  # [NQ, E]
    if cnt.max() > CAP:
        raise RuntimeError(f"routing capacity exceeded: {cnt.max()} > {CAP}")

    utri = np.triu(np.ones((P, P), np.float32), k=1)
    ones = np.ones((P, P), np.float32)
    ident = np.eye(P, dtype=np.float32)
    tidb = (np.arange(E)[None, :] * P + np.arange(P)[:, None]).astype(np.float32)
    sr = np.broadcast_to(np.arange(CT * P, dtype=np.float32)[None, :],
                         (P, CT * P)).copy()

    maps = []
    for e in range(NCORES):
        perm = [e] + [j for j in range(E) if j != e]
        gwT = np.ascontiguousarray(gate_w[perm].T)
        maps.append({
            "xT": xT,
            "x": xpad,
            "gwT": gwT,
            "w1T": np.ascontiguousarray(np.asarray(w1[e], np.float32).T),
            "w3T": np.ascontiguousarray(np.asarray(w3[e], np.float32).T),
            "w2T": np.ascontiguousarray(np.asarray(w2[e], np.float32).T),
            "utri": utri,
            "ones": ones,
            "ident": ident,
            "tidb": tidb,
            "sr": sr,
        })
    return maps


def run(x, gate_w, w1, w3, w2, trace=False, trace_cores=None):
    nc = _get_nc()
    maps = _in_maps(x, gate_w, w1, w3, w2)
    res = run_bass_kernel_spmd(
        nc, maps, core_ids=list(range(NCORES)), trace=trace,
        trace_cores=trace_cores)
    # core r's output block q (128 rows) holds tokens [1024q + 128r, +128)
    y = np.empty((T, D), dtype=np.float32)
    for r in range(NCORES):
        yr = res.results[r]["y"]
        for q in range(NQ):
            t0 = q * RT + r * RSH
            y[t0:t0 + RSH] = yr[q * RSH:(q + 1) * RSH]
    return y, res


def kernel(x, gate_w, w1, w3, w2):
    y, _ = run(x, gate_w, w1, w3, w2, trace=False)
    return y.astype(np.float32)
